# revision 1
# baseline (speedup 1.0000x reference)
"""NF4 dequantization kernel for Trainium2 (8 NeuronCores, tensor-parallel).

Computes: out[g*32+r, n] = nf4_poly(quants[g, r, n]) * scales[g, 0, n]
where nf4_poly is a fixed degree-5 polynomial and quants hold 4-bit codes
(0..15) stored as int32.

Strategy
--------
- Shard along the last (N) axis across 8 cores (no communication needed).
- The quintic is factored over the reals:
      p(x) = c5 * (x - g) * (x^2 + a1 x + b1) * (x^2 + a2 x + b2)
  (one real root g, two complex-conjugate pairs -> well-conditioned
  quadratics, no cancellation for x in [0, 15]).
- Two custom DVE (vector-engine) instructions evaluate the whole thing,
  reading the int32 codes directly (DVE converts on read):
      op1: u   = (x^2 + a1 x + b1) * s'          s' = c5 * scales,
                                                 broadcast via 0-stride AP
      op2: out = u * (x - g) * (x^2 + a2 x + b2)
  => 2 elementwise passes total; the kernel is DMA/HBM-bound.
- Layout: partitions = quant groups (128 at a time), free dim = (4 rows of
  the group) x (1024 N-columns of this core's shard) = 16 KiB contiguous
  4 KiB DMA chunks.
"""

import numpy as np

import concourse.bacc as bacc
import concourse.mybir as mybir
import concourse.tile as tile
import concourse.dve_ops as dve_ops
from concourse.dve_spec import Spec, Src0, Src1, C0, C1, C2, sq, lower, _has_src1
from concourse.dve_uop import DveOpSpec
from concourse import bass_utils

# ---------------------------------------------------------------- constants
# reference polynomial (BIG_POLYNOMIAL=False NF4 approximation)
_C5 = 1.82943132356953e-05
# real factorization of the monic quintic p(x)/c5 (computed in float64):
#   (x - GAMMA) (x^2 + A1 x + B1) (x^2 + A2 x + B2)
_GAMMA = 7.08749475940335
_A1, _B1 = -27.553653740000001, 220.05216916806501
_A2, _B2 = -2.85016274, 34.843717690337314

_NCORES = 8
_G, _GS, _N = 256, 32, 8192          # full input shape
_NS = _N // _NCORES                  # 1024 columns per core
_RS = 4                              # group-rows per tile
_GB = 128                            # groups per partition block


def _register_op(name, spec):
    """Append a custom DVE op to the concourse registry (idempotent)."""
    for op in dve_ops.OPS:
        if op.name == name:
            return op
    row = dve_ops._CUSTOM_DVE_ROW_BASE + len(dve_ops.OPS)
    assert row < 0x20, "custom DVE opcode rows exhausted"
    shas = {
        ver: DveOpSpec(
            name=name, opcode=row, uops=lower(spec, ver=ver), rd1_en=_has_src1(spec)
        ).sha(ver)
        for ver in ("v3", "v4")
    }
    op = dve_ops.DveOp(name, spec, subdim=False, uops_sha=shas)
    dve_ops.OPS.append(op)
    dve_ops.CUSTOM_DVE_SPECS[name] = spec
    dve_ops._SUB_OPCODE_FOR_NAME[name] = row
    return op


def _make_ops():
    op1 = _register_op(
        "NF4_STAGE1_ANT",
        Spec(
            body=(sq(Src0) + Src0 * C0 + C1) * Src1,
            reference=lambda in0, in1, s0, s1, imm2: (in0 * in0 + s0 * in0 + s1)
            * in1,
        ),
    )
    op2 = _register_op(
        "NF4_STAGE2_ANT",
        Spec(
            body=Src0 * (Src1 - C2) * (sq(Src1) + Src1 * C0 + C1),
            reference=lambda in0, in1, s0, s1, imm2: in0
            * (in1 - imm2)
            * (in1 * in1 + s0 * in1 + s1),
        ),
    )
    return op1, op2


_NC_CACHE = {}


def _build_module():
    """Build + compile the per-core Bass module (identical on all cores)."""
    if "nc" in _NC_CACHE:
        return _NC_CACHE["nc"]

    op1, op2 = _make_ops()
    nc = bacc.Bacc(
        "TRN2",
        target_bir_lowering=False,
        debug=False,
        enable_asserts=False,
        num_devices=_NCORES,
    )
    q_d = nc.dram_tensor(
        "quants", [_G, _GS, _NS], mybir.dt.int32, kind="ExternalInput"
    ).ap()
    s_d = nc.dram_tensor(
        "scales", [_G, _NS], mybir.dt.float32, kind="ExternalInput"
    ).ap()
    o_d = nc.dram_tensor(
        "out", [_G, _GS, _NS], mybir.dt.float32, kind="ExternalOutput"
    ).ap()

    fd = _RS * _NS
    with tile.TileContext(nc) as tc:
        with (
            tc.tile_pool(name="sc", bufs=2) as sc_pool,
            tc.tile_pool(name="q", bufs=3) as q_pool,
            tc.tile_pool(name="u", bufs=2) as u_pool,
            tc.tile_pool(name="o", bufs=3) as o_pool,
        ):
            for gb in range(_G // _GB):
                gsl = slice(gb * _GB, (gb + 1) * _GB)
                s_raw = sc_pool.tile([_GB, _NS], mybir.dt.float32, tag="sraw")
                nc.sync.dma_start(s_raw[:], s_d[gsl, :])
                s_p = sc_pool.tile([_GB, _NS], mybir.dt.float32, tag="sp")
                # s' = c5 * scales (on the otherwise-idle scalar engine)
                nc.scalar.mul(s_p[:], s_raw[:], _C5)
                s_b = s_p[:, None, :].broadcast_to([_GB, _RS, _NS])

                for rc in range(_GS // _RS):
                    rsl = slice(rc * _RS, (rc + 1) * _RS)
                    qt = q_pool.tile([_GB, fd], mybir.dt.int32)
                    nc.sync.dma_start(
                        qt[:].rearrange("p (r n) -> p r n", r=_RS),
                        q_d[gsl, rsl, :],
                    )
                    ut = u_pool.tile([_GB, fd], mybir.dt.float32)
                    nc.vector._custom_dve(
                        op1, out=ut[:], in0=qt[:], in1=s_b, s0=_A1, s1=_B1
                    )
                    ot = o_pool.tile([_GB, fd], mybir.dt.float32)
                    nc.vector._custom_dve(
                        op2, out=ot[:], in0=ut[:], in1=qt[:],
                        s0=_A2, s1=_B2, imm2=_GAMMA,
                    )
                    # store on the ACT HWDGE ring so loads/stores overlap
                    nc.scalar.dma_start(
                        o_d[gsl, rsl, :],
                        ot[:].rearrange("p (r n) -> p r n", r=_RS),
                    )

    nc.compile()
    _NC_CACHE["nc"] = nc
    return nc


def kernel(quants: np.ndarray, scales: np.ndarray, **_) -> np.ndarray:
    assert quants.shape == (_G, _GS, _N) and scales.shape == (_G, 1, _N)
    nc = _build_module()

    in_maps = []
    for i in range(_NCORES):
        csl = slice(i * _NS, (i + 1) * _NS)
        in_maps.append(
            {
                "quants": np.ascontiguousarray(quants[:, :, csl], dtype=np.int32),
                "scales": np.ascontiguousarray(
                    scales[:, 0, csl], dtype=np.float32
                ),
            }
        )

    res = bass_utils.run_bass_kernel_spmd(
        nc, in_maps, core_ids=list(range(_NCORES))
    )
    shards = [r["out"].reshape(_G * _GS, _NS) for r in res.results]
    return np.concatenate(shards, axis=1)


if __name__ == "__main__":
    rng = np.random.default_rng(0)
    q = rng.integers(0, 16, (_G, _GS, _N)).astype(np.int32)
    s = rng.random((_G, 1, _N)).astype(np.float32)
    out = kernel(quants=q, scales=s)
    print("out", out.shape, out.dtype, out[0, :4])


# revision 4
# speedup vs baseline: 2.5846x; 2.5846x over previous
"""NF4 dequantization kernel for Trainium2 (8 NeuronCores, tensor-parallel).

Computes: out[g*32+r, n] = nf4_poly(quants[g, r, n]) * scales[g, 0, n]
where nf4_poly is a fixed degree-5 polynomial and quants hold 4-bit codes
(0..15) stored as int32.

Strategy
--------
- Shard along the last (N) axis across 8 cores (no communication needed).
- The quintic is factored over the reals:
      p(x) = c5 * (x - g) * (x^2 + a1 x + b1) * (x^2 + a2 x + b2)
  (one real root g, two complex-conjugate pairs -> well-conditioned
  quadratics, no cancellation for x in [0, 15]).
- Two custom DVE (vector-engine) instructions evaluate the whole thing,
  reading the int32 codes directly (DVE converts on read):
      op1: u   = (x^2 + a1 x + b1) * s'          s' = c5 * scales,
                                                 broadcast via 0-stride AP
      op2: out = u * (x - g) * (x^2 + a2 x + b2)
  => 2 elementwise passes total; the kernel is DMA/HBM-bound.
- Layout: partitions = quant groups (128 at a time), free dim = (4 rows of
  the group) x (1024 N-columns of this core's shard) = 16 KiB contiguous
  4 KiB DMA chunks.
"""

import numpy as np

import concourse.bacc as bacc
import concourse.mybir as mybir
import concourse.tile as tile
import concourse.dve_ops as dve_ops
from concourse.dve_spec import Spec, Src0, Src1, C0, C1, C2, sq, lower, _has_src1
from concourse.dve_uop import DveOpSpec
from concourse import bass_utils

# ---------------------------------------------------------------- constants
# reference polynomial (BIG_POLYNOMIAL=False NF4 approximation)
_C5 = 1.82943132356953e-05
# real factorization of the monic quintic p(x)/c5 (computed in float64):
#   (x - GAMMA) (x^2 + A1 x + B1) (x^2 + A2 x + B2)
_GAMMA = 7.08749475940335
_A1, _B1 = -27.553653740000001, 220.05216916806501
_A2, _B2 = -2.85016274, 34.843717690337314

_NCORES = 8
_G, _GS, _N = 256, 32, 8192          # full input shape
_NS = _N // _NCORES                  # 1024 columns per core
_RS = 4                              # group-rows per tile
_GB = 128                            # groups per partition block


def _register_op(name, spec):
    """Append a custom DVE op to the concourse registry (idempotent)."""
    for op in dve_ops.OPS:
        if op.name == name:
            return op
    row = dve_ops._CUSTOM_DVE_ROW_BASE + len(dve_ops.OPS)
    assert row < 0x20, "custom DVE opcode rows exhausted"
    shas = {
        ver: DveOpSpec(
            name=name, opcode=row, uops=lower(spec, ver=ver), rd1_en=_has_src1(spec)
        ).sha(ver)
        for ver in ("v3", "v4")
    }
    op = dve_ops.DveOp(name, spec, subdim=False, uops_sha=shas)
    dve_ops.OPS.append(op)
    dve_ops.CUSTOM_DVE_SPECS[name] = spec
    dve_ops._SUB_OPCODE_FOR_NAME[name] = row
    return op


def _make_ops():
    op1 = _register_op(
        "NF4_STAGE1_ANT",
        Spec(
            body=(sq(Src0) + Src0 * C0 + C1) * Src1,
            reference=lambda in0, in1, s0, s1, imm2: (in0 * in0 + s0 * in0 + s1)
            * in1,
        ),
    )
    op2 = _register_op(
        "NF4_STAGE2_ANT",
        Spec(
            body=Src0 * (Src1 - C2) * (sq(Src1) + Src1 * C0 + C1),
            reference=lambda in0, in1, s0, s1, imm2: in0
            * (in1 - imm2)
            * (in1 * in1 + s0 * in1 + s1),
        ),
    )
    return op1, op2


_NC_CACHE = {}


def _build_module(_repeat=1):
    """Build + compile the per-core Bass module (identical on all cores).

    `_repeat` re-runs the whole loop nest N times over the same data —
    used only by benchmarking to measure marginal per-work time."""
    if _repeat in _NC_CACHE:
        return _NC_CACHE[_repeat]

    op1, op2 = _make_ops()
    nc = bacc.Bacc(
        "TRN2",
        target_bir_lowering=False,
        debug=False,
        enable_asserts=False,
        num_devices=_NCORES,
    )
    q_d = nc.dram_tensor(
        "quants", [_G, _GS, _NS], mybir.dt.int32, kind="ExternalInput"
    ).ap()
    s_d = nc.dram_tensor(
        "scales", [_G, _NS], mybir.dt.float32, kind="ExternalInput"
    ).ap()
    o_d = nc.dram_tensor(
        "out", [_G, _GS, _NS], mybir.dt.float32, kind="ExternalOutput"
    ).ap()

    fd = _RS * _NS
    with tile.TileContext(nc) as tc:
        with (
            tc.tile_pool(name="sc", bufs=2) as sc_pool,
            tc.tile_pool(name="q", bufs=3) as q_pool,
            tc.tile_pool(name="u", bufs=2) as u_pool,
            tc.tile_pool(name="o", bufs=3) as o_pool,
        ):
            for gb in [g for g in range(_G // _GB) for _ in range(_repeat)]:
                gsl = slice(gb * _GB, (gb + 1) * _GB)
                s_raw = sc_pool.tile([_GB, _NS], mybir.dt.float32, tag="sraw")
                nc.sync.dma_start(s_raw[:], s_d[gsl, :])
                s_p = sc_pool.tile([_GB, _NS], mybir.dt.float32, tag="sp")
                # s' = c5 * scales (on the otherwise-idle scalar engine)
                nc.scalar.mul(s_p[:], s_raw[:], _C5)
                s_b = s_p[:, None, :].broadcast_to([_GB, _RS, _NS])

                for rc in range(_GS // _RS):
                    rsl = slice(rc * _RS, (rc + 1) * _RS)
                    qt = q_pool.tile([_GB, fd], mybir.dt.int32)
                    nc.sync.dma_start(
                        qt[:].rearrange("p (r n) -> p r n", r=_RS),
                        q_d[gsl, rsl, :],
                    )
                    ut = u_pool.tile([_GB, fd], mybir.dt.float32)
                    nc.vector._custom_dve(
                        op1, out=ut[:], in0=qt[:], in1=s_b, s0=_A1, s1=_B1
                    )
                    ot = o_pool.tile([_GB, fd], mybir.dt.float32)
                    nc.vector._custom_dve(
                        op2, out=ot[:], in0=ut[:], in1=qt[:],
                        s0=_A2, s1=_B2, imm2=_GAMMA,
                    )
                    # store on the ACT HWDGE ring so loads/stores overlap
                    nc.scalar.dma_start(
                        o_d[gsl, rsl, :],
                        ot[:].rearrange("p (r n) -> p r n", r=_RS),
                    )

    nc.compile()
    _NC_CACHE[_repeat] = nc
    return nc


def kernel(quants: np.ndarray, scales: np.ndarray, **_) -> np.ndarray:
    assert quants.shape == (_G, _GS, _N) and scales.shape == (_G, 1, _N)
    nc = _build_module()

    in_maps = []
    for i in range(_NCORES):
        csl = slice(i * _NS, (i + 1) * _NS)
        in_maps.append(
            {
                "quants": np.ascontiguousarray(quants[:, :, csl], dtype=np.int32),
                "scales": np.ascontiguousarray(
                    scales[:, 0, csl], dtype=np.float32
                ),
            }
        )

    res = bass_utils.run_bass_kernel_spmd(
        nc, in_maps, core_ids=list(range(_NCORES))
    )
    shards = [r["out"].reshape(_G * _GS, _NS) for r in res.results]
    return np.concatenate(shards, axis=1)


if __name__ == "__main__":
    rng = np.random.default_rng(0)
    q = rng.integers(0, 16, (_G, _GS, _N)).astype(np.int32)
    s = rng.random((_G, 1, _N)).astype(np.float32)
    out = kernel(quants=q, scales=s)
    print("out", out.shape, out.dtype, out[0, :4])


# revision 5
# speedup vs baseline: 2.7386x; 1.0596x over previous
"""NF4 dequantization kernel for Trainium2 (8 NeuronCores, tensor-parallel).

Computes: out[g*32+r, n] = nf4_poly(quants[g, r, n]) * scales[g, 0, n]
where nf4_poly is a fixed degree-5 polynomial and quants hold 4-bit codes
(0..15) stored as int32.

Strategy
--------
- Shard along the last (N) axis across 8 cores (no communication needed).
- The quintic is factored over the reals:
      p(x) = c5 * (x - g) * (x^2 + a1 x + b1) * (x^2 + a2 x + b2)
  (one real root g, two complex-conjugate pairs -> well-conditioned
  quadratics, no cancellation for x in [0, 15]).
- Two custom DVE (vector-engine) instructions evaluate the whole thing,
  reading the int32 codes directly (DVE converts on read):
      op1: u   = (x^2 + a1 x + b1) * s'          s' = c5 * scales,
                                                 broadcast via 0-stride AP
      op2: out = u * (x - g) * (x^2 + a2 x + b2)
  => 2 elementwise passes total; the kernel is DMA/HBM-bound.
- Layout: partitions = quant groups (128 at a time), free dim = (4 rows of
  the group) x (1024 N-columns of this core's shard) = 16 KiB contiguous
  4 KiB DMA chunks.
"""

import numpy as np

import concourse.bacc as bacc
import concourse.mybir as mybir
import concourse.tile as tile
import concourse.dve_ops as dve_ops
from concourse.dve_spec import Spec, Src0, Src1, C0, C1, C2, sq, lower, _has_src1
from concourse.dve_uop import DveOpSpec
from concourse import bass_utils

# ---------------------------------------------------------------- constants
# reference polynomial (BIG_POLYNOMIAL=False NF4 approximation)
_C5 = 1.82943132356953e-05
# real factorization of the monic quintic p(x)/c5 (computed in float64):
#   (x - GAMMA) (x^2 + A1 x + B1) (x^2 + A2 x + B2)
_GAMMA = 7.08749475940335
_A1, _B1 = -27.553653740000001, 220.05216916806501
_A2, _B2 = -2.85016274, 34.843717690337314

_NCORES = 8
_G, _GS, _N = 256, 32, 8192          # full input shape
_NS = _N // _NCORES                  # 1024 columns per core
_RS = 4                              # group-rows per tile
_GB = 128                            # groups per partition block


def _register_op(name, spec):
    """Append a custom DVE op to the concourse registry (idempotent)."""
    for op in dve_ops.OPS:
        if op.name == name:
            return op
    row = dve_ops._CUSTOM_DVE_ROW_BASE + len(dve_ops.OPS)
    assert row < 0x20, "custom DVE opcode rows exhausted"
    shas = {
        ver: DveOpSpec(
            name=name, opcode=row, uops=lower(spec, ver=ver), rd1_en=_has_src1(spec)
        ).sha(ver)
        for ver in ("v3", "v4")
    }
    op = dve_ops.DveOp(name, spec, subdim=False, uops_sha=shas)
    dve_ops.OPS.append(op)
    dve_ops.CUSTOM_DVE_SPECS[name] = spec
    dve_ops._SUB_OPCODE_FOR_NAME[name] = row
    return op


def _make_ops():
    op1 = _register_op(
        "NF4_STAGE1_ANT",
        Spec(
            body=(sq(Src0) + Src0 * C0 + C1) * Src1,
            reference=lambda in0, in1, s0, s1, imm2: (in0 * in0 + s0 * in0 + s1)
            * in1,
        ),
    )
    op2 = _register_op(
        "NF4_STAGE2_ANT",
        Spec(
            body=Src0 * (Src1 - C2) * (sq(Src1) + Src1 * C0 + C1),
            reference=lambda in0, in1, s0, s1, imm2: in0
            * (in1 - imm2)
            * (in1 * in1 + s0 * in1 + s1),
        ),
    )
    return op1, op2


_NC_CACHE = {}


def _build_module(_repeat=1):
    """Build + compile the per-core Bass module (identical on all cores).

    `_repeat` re-runs the whole loop nest N times over the same data —
    used only by benchmarking to measure marginal per-work time."""
    if _repeat in _NC_CACHE:
        return _NC_CACHE[_repeat]

    op1, op2 = _make_ops()
    nc = bacc.Bacc(
        "TRN2",
        target_bir_lowering=False,
        debug=False,
        enable_asserts=False,
        num_devices=_NCORES,
    )
    q_d = nc.dram_tensor(
        "quants", [_G, _GS, _NS], mybir.dt.int32, kind="ExternalInput"
    ).ap()
    s_d = nc.dram_tensor(
        "scales", [_G, _NS], mybir.dt.float32, kind="ExternalInput"
    ).ap()
    o_d = nc.dram_tensor(
        "out", [_G, _GS, _NS], mybir.dt.float32, kind="ExternalOutput"
    ).ap()

    fd = _RS * _NS
    with tile.TileContext(nc) as tc:
        with (
            tc.tile_pool(name="sc", bufs=2) as sc_pool,
            tc.tile_pool(name="q", bufs=3) as q_pool,
            tc.tile_pool(name="u", bufs=2) as u_pool,
            tc.tile_pool(name="o", bufs=3) as o_pool,
        ):
            for gb in [g for g in range(_G // _GB) for _ in range(_repeat)]:
                gsl = slice(gb * _GB, (gb + 1) * _GB)
                s_raw = sc_pool.tile([_GB, _NS], mybir.dt.float32, tag="sraw")
                nc.sync.dma_start(s_raw[:], s_d[gsl, :])
                s_p = sc_pool.tile([_GB, _NS], mybir.dt.float32, tag="sp")
                # s' = c5 * scales (on the otherwise-idle scalar engine)
                nc.scalar.mul(s_p[:], s_raw[:], _C5)
                s_b = s_p[:, None, :].broadcast_to([_GB, _RS, _NS])

                for rc in range(_GS // _RS):
                    rsl = slice(rc * _RS, (rc + 1) * _RS)
                    qt = q_pool.tile([_GB, fd], mybir.dt.int32)
                    nc.sync.dma_start(
                        qt[:].rearrange("p (r n) -> p r n", r=_RS),
                        q_d[gsl, rsl, :],
                    )
                    ut = u_pool.tile([_GB, fd], mybir.dt.float32)
                    nc.vector._custom_dve(
                        op1, out=ut[:], in0=qt[:], in1=s_b, s0=_A1, s1=_B1
                    )
                    ot = o_pool.tile([_GB, fd], mybir.dt.float32)
                    nc.vector._custom_dve(
                        op2, out=ot[:], in0=ut[:], in1=qt[:],
                        s0=_A2, s1=_B2, imm2=_GAMMA,
                    )
                    # store on the ACT HWDGE ring so loads/stores overlap
                    nc.scalar.dma_start(
                        o_d[gsl, rsl, :],
                        ot[:].rearrange("p (r n) -> p r n", r=_RS),
                    )

    nc.compile()
    _NC_CACHE[_repeat] = nc
    return nc


def kernel(quants: np.ndarray, scales: np.ndarray, **_) -> np.ndarray:
    quants = np.asarray(quants)
    scales = np.asarray(scales)
    assert quants.shape == (_G, _GS, _N) and scales.shape == (_G, 1, _N)
    nc = _build_module()

    in_maps = []
    for i in range(_NCORES):
        csl = slice(i * _NS, (i + 1) * _NS)
        in_maps.append(
            {
                "quants": np.ascontiguousarray(quants[:, :, csl], dtype=np.int32),
                "scales": np.ascontiguousarray(
                    scales[:, 0, csl], dtype=np.float32
                ),
            }
        )

    res = bass_utils.run_bass_kernel_spmd(
        nc, in_maps, core_ids=list(range(_NCORES))
    )
    shards = [r["out"].reshape(_G * _GS, _NS) for r in res.results]
    return np.concatenate(shards, axis=1)


if __name__ == "__main__":
    rng = np.random.default_rng(0)
    q = rng.integers(0, 16, (_G, _GS, _N)).astype(np.int32)
    s = rng.random((_G, 1, _N)).astype(np.float32)
    out = kernel(quants=q, scales=s)
    print("out", out.shape, out.dtype, out[0, :4])


# revision 6
# speedup vs baseline: 2.8679x; 1.0472x over previous
"""NF4 dequantization kernel for Trainium2 (8 NeuronCores, tensor-parallel).

Computes: out[g*32+r, n] = nf4_poly(quants[g, r, n]) * scales[g, 0, n]
where nf4_poly is a fixed degree-5 polynomial and quants hold 4-bit codes
(0..15) stored as int32.

Strategy
--------
- Shard along the last (N) axis across 8 cores (no communication needed).
- The quintic is factored over the reals:
      p(x) = c5 * (x - g) * (x^2 + a1 x + b1) * (x^2 + a2 x + b2)
  (one real root g, two complex-conjugate pairs -> well-conditioned
  quadratics, no cancellation for x in [0, 15]).
- Two custom DVE (vector-engine) instructions evaluate the whole thing,
  reading the int32 codes directly (DVE converts on read):
      op1: u   = (x^2 + a1 x + b1) * s'          s' = c5 * scales,
                                                 broadcast via 0-stride AP
      op2: out = u * (x - g) * (x^2 + a2 x + b2)
  => 2 elementwise passes total; the kernel is DMA/HBM-bound.
- Layout: partitions = quant groups (128 at a time), free dim = (4 rows of
  the group) x (1024 N-columns of this core's shard) = 16 KiB contiguous
  4 KiB DMA chunks.
"""

import numpy as np

import concourse.bacc as bacc
import concourse.mybir as mybir
import concourse.tile as tile
import concourse.dve_ops as dve_ops
from concourse.dve_spec import Spec, Src0, Src1, C0, C1, C2, sq, lower, _has_src1
from concourse.dve_uop import DveOpSpec
from concourse import bass_utils

# ---------------------------------------------------------------- constants
# Real factorization of the reference quintic:
#   p(x) = C5 (x - GAMMA) (x^2 + A1 x + B1) (x^2 + A2 x + B2)
# Constants are fp32-representable, coordinate-descent-tuned (in exact fp32
# pipeline emulation, which is bit-identical to the DVE) against the fp32
# Horner reference so the factored evaluation tracks it to ~4e-7 rel.
_C5 = 1.829428583732806e-05
_GAMMA = 7.087499141693115
_A1, _B1 = -27.55365562438965, 220.05215454101562
_A2, _B2 = -2.8501572608947754, 34.843746185302734

_NCORES = 8
_G, _GS, _N = 256, 32, 8192          # full input shape
_NS = _N // _NCORES                  # 1024 columns per core
_RS = 4                              # group-rows per tile
_GB = 128                            # groups per partition block


def _register_op(name, spec):
    """Append a custom DVE op to the concourse registry (idempotent)."""
    for op in dve_ops.OPS:
        if op.name == name:
            return op
    row = dve_ops._CUSTOM_DVE_ROW_BASE + len(dve_ops.OPS)
    assert row < 0x20, "custom DVE opcode rows exhausted"
    shas = {
        ver: DveOpSpec(
            name=name, opcode=row, uops=lower(spec, ver=ver), rd1_en=_has_src1(spec)
        ).sha(ver)
        for ver in ("v3", "v4")
    }
    op = dve_ops.DveOp(name, spec, subdim=False, uops_sha=shas)
    dve_ops.OPS.append(op)
    dve_ops.CUSTOM_DVE_SPECS[name] = spec
    dve_ops._SUB_OPCODE_FOR_NAME[name] = row
    return op


def _make_ops():
    op1 = _register_op(
        "NF4_STAGE1_ANT",
        Spec(
            body=(sq(Src0) + Src0 * C0 + C1) * Src1,
            reference=lambda in0, in1, s0, s1, imm2: (in0 * in0 + s0 * in0 + s1)
            * in1,
        ),
    )
    op2 = _register_op(
        "NF4_STAGE2_ANT",
        Spec(
            body=Src0 * (Src1 - C2) * (sq(Src1) + Src1 * C0 + C1),
            reference=lambda in0, in1, s0, s1, imm2: in0
            * (in1 - imm2)
            * (in1 * in1 + s0 * in1 + s1),
        ),
    )
    return op1, op2


_NC_CACHE = {}


def _build_module(_repeat=1):
    """Build + compile the per-core Bass module (identical on all cores).

    `_repeat` re-runs the whole loop nest N times over the same data —
    used only by benchmarking to measure marginal per-work time."""
    if _repeat in _NC_CACHE:
        return _NC_CACHE[_repeat]

    op1, op2 = _make_ops()
    nc = bacc.Bacc(
        "TRN2",
        target_bir_lowering=False,
        debug=False,
        enable_asserts=False,
        num_devices=_NCORES,
    )
    q_d = nc.dram_tensor(
        "quants", [_G, _GS, _NS], mybir.dt.int32, kind="ExternalInput"
    ).ap()
    s_d = nc.dram_tensor(
        "scales", [_G, _NS], mybir.dt.float32, kind="ExternalInput"
    ).ap()
    o_d = nc.dram_tensor(
        "out", [_G, _GS, _NS], mybir.dt.float32, kind="ExternalOutput"
    ).ap()

    fd = _RS * _NS
    with tile.TileContext(nc) as tc:
        with (
            tc.tile_pool(name="sc", bufs=2) as sc_pool,
            tc.tile_pool(name="q", bufs=3) as q_pool,
            tc.tile_pool(name="u", bufs=2) as u_pool,
            tc.tile_pool(name="o", bufs=3) as o_pool,
        ):
            for gb in [g for g in range(_G // _GB) for _ in range(_repeat)]:
                gsl = slice(gb * _GB, (gb + 1) * _GB)
                s_raw = sc_pool.tile([_GB, _NS], mybir.dt.float32, tag="sraw")
                nc.sync.dma_start(s_raw[:], s_d[gsl, :])
                s_p = sc_pool.tile([_GB, _NS], mybir.dt.float32, tag="sp")
                # s' = c5 * scales (on the otherwise-idle scalar engine)
                nc.scalar.mul(s_p[:], s_raw[:], _C5)
                s_b = s_p[:, None, :].broadcast_to([_GB, _RS, _NS])

                for rc in range(_GS // _RS):
                    rsl = slice(rc * _RS, (rc + 1) * _RS)
                    qt = q_pool.tile([_GB, fd], mybir.dt.int32)
                    nc.sync.dma_start(
                        qt[:].rearrange("p (r n) -> p r n", r=_RS),
                        q_d[gsl, rsl, :],
                    )
                    ut = u_pool.tile([_GB, fd], mybir.dt.float32)
                    nc.vector._custom_dve(
                        op1, out=ut[:], in0=qt[:], in1=s_b, s0=_A1, s1=_B1
                    )
                    ot = o_pool.tile([_GB, fd], mybir.dt.float32)
                    nc.vector._custom_dve(
                        op2, out=ot[:], in0=ut[:], in1=qt[:],
                        s0=_A2, s1=_B2, imm2=_GAMMA,
                    )
                    # store on the ACT HWDGE ring so loads/stores overlap
                    nc.scalar.dma_start(
                        o_d[gsl, rsl, :],
                        ot[:].rearrange("p (r n) -> p r n", r=_RS),
                    )

    nc.compile()
    _NC_CACHE[_repeat] = nc
    return nc


def kernel(quants: np.ndarray, scales: np.ndarray, **_) -> np.ndarray:
    quants = np.asarray(quants)
    scales = np.asarray(scales)
    assert quants.shape == (_G, _GS, _N) and scales.shape == (_G, 1, _N)
    nc = _build_module()

    in_maps = []
    for i in range(_NCORES):
        csl = slice(i * _NS, (i + 1) * _NS)
        in_maps.append(
            {
                "quants": np.ascontiguousarray(quants[:, :, csl], dtype=np.int32),
                "scales": np.ascontiguousarray(
                    scales[:, 0, csl], dtype=np.float32
                ),
            }
        )

    res = bass_utils.run_bass_kernel_spmd(
        nc, in_maps, core_ids=list(range(_NCORES))
    )
    shards = [r["out"].reshape(_G * _GS, _NS) for r in res.results]
    return np.concatenate(shards, axis=1)


if __name__ == "__main__":
    rng = np.random.default_rng(0)
    q = rng.integers(0, 16, (_G, _GS, _N)).astype(np.int32)
    s = rng.random((_G, 1, _N)).astype(np.float32)
    out = kernel(quants=q, scales=s)
    print("out", out.shape, out.dtype, out[0, :4])
